# revision 13
# baseline (speedup 1.0000x reference)
"""Causal self-attention (B=4, T=2048, D=1024, H=16, hd=64) on 8 trn2 NeuronCores.

Sharding: data parallel over batch (4) x tensor parallel over heads (2 groups
of 8). Core c handles batch c//2 and heads (c%2)*8 .. (c%2)*8+8.
Wq/Wk/Wv are column-parallel by head group, Wo row-parallel; the pair of
cores sharing a batch produce partial outputs that are summed on the host.

On-device layout (per core) is fully "transposed": projections produce
Q^T, K^T [512, 2048] and V [2048, 512], scores are computed as
S^T = K Q^T (j=key on partitions, i=query on free dim), softmax uses
exp without max subtraction (scores are O(6) here), the denominator
comes for free from a ones-column appended to V, and attention output
O^T [hd, T] feeds the row-parallel out-projection directly as lhsT.

v2 schedule: the scalar engine's exp stream (160 x ~1.1us) is the pacer.
Everything else (Q/K/V projections, out-projection) is emitted as fillers
inside the attention j-tile loop so exp starts ~10us into the run and the
PE never idles waiting for it. Exp and the S matmuls are trimmed to the
causally-valid query range of each diagonal tile.
"""

import contextlib
import ctypes
import sys
import types

import numpy as np

B, T, D = 4, 2048, 1024
H_TOT, HD = 16, 64
SCALE = HD ** -0.5
P = 128
NH = 8            # heads per core
QD = NH * HD      # 512, projected dim per core
KT = D // P       # 8 contraction tiles for projections
MT = QD // P      # 4 qdim tiles
TT = T // P       # 16 token tiles
ACH = 512         # token chunk; PSUM bank caps matmul N at 512
NACH = T // ACH   # 4
ICH = 512         # attention query chunk
NIC = T // ICH    # 4

_PROGRAM = None  # compiled program cache — build once per process


def _install_ntff_hook():
    """antenv.axon_hooks is missing in this image; recreate it so
    run_bass_kernel_spmd(trace=True) can profile. Harmless if unused."""
    if "antenv.axon_hooks" in sys.modules:
        return
    try:
        import antenv
    except ImportError:
        return
    mod = types.ModuleType("antenv.axon_hooks")
    _hook = [None]
    mod.set_axon_ntff_profile_hook = lambda h: _hook.__setitem__(0, h)
    mod.get_axon_ntff_profile_hook = lambda: _hook[0]
    antenv.axon_hooks = mod
    sys.modules["antenv.axon_hooks"] = mod
    try:
        lib = ctypes.CDLL("/opt/axon/libaxon_pjrt.so")
        if not hasattr(lib, "axon_start_nrt_profile"):
            return
        lib.axon_start_nrt_profile.argtypes = [
            ctypes.POINTER(ctypes.c_int64), ctypes.c_size_t]
        lib.axon_start_nrt_profile.restype = ctypes.c_int64
        lib.axon_stop_nrt_profile.argtypes = [ctypes.c_char_p]
        lib.axon_stop_nrt_profile.restype = ctypes.c_int64

        @contextlib.contextmanager
        def _hookfn(output_dir, device_ids):
            import jax
            jax.devices()
            if device_ids:
                ids = (ctypes.c_int64 * len(device_ids))(*device_ids)
                rc = lib.axon_start_nrt_profile(ids, len(device_ids))
            else:
                rc = lib.axon_start_nrt_profile(None, 0)
            if rc != 0:
                raise RuntimeError(f"axon_start_nrt_profile rc={rc}")
            try:
                yield
            finally:
                n = lib.axon_stop_nrt_profile(str(output_dir).encode())
                print(f"profile: {n} file(s) written to {output_dir}")

        mod.set_axon_ntff_profile_hook(_hookfn)
    except OSError:
        pass


def _build_program():
    from contextlib import ExitStack

    import concourse.tile as tile
    from concourse import bacc, mybir

    F32 = mybir.dt.float32
    BF16 = mybir.dt.bfloat16
    AF = mybir.ActivationFunctionType
    ALU = mybir.AluOpType

    nc = bacc.Bacc("TRN2", target_bir_lowering=False, debug=False,
                   num_devices=8)

    # all tensor inputs arrive pre-arranged in SBUF layout [128, k, n]
    # (host does the transpose) so every DMA is long contiguous runs
    xT_d = nc.dram_tensor("xT", [P, KT * T], BF16, kind="ExternalInput").ap()
    wq_d = nc.dram_tensor("wq", [P, KT * QD], BF16, kind="ExternalInput").ap()
    wk_d = nc.dram_tensor("wk", [P, KT * QD], BF16, kind="ExternalInput").ap()
    wv_d = nc.dram_tensor("wv", [P, KT * QD], BF16, kind="ExternalInput").ap()
    wo_d = nc.dram_tensor("wo", [P, MT * D], BF16, kind="ExternalInput").ap()
    bq_d = nc.dram_tensor("bq", [P, MT], F32, kind="ExternalInput").ap()
    bk_d = nc.dram_tensor("bk", [P, MT], F32, kind="ExternalInput").ap()
    bvb_d = nc.dram_tensor("bvb", [P, QD], F32, kind="ExternalInput").ap()
    msk_d = nc.dram_tensor("msk", [P, P], BF16, kind="ExternalInput").ap()
    out_d = nc.dram_tensor("out", [T, D], F32, kind="ExternalOutput").ap()

    xT_k = xT_d.rearrange("p (k t) -> p k t", k=KT)      # [128, 8, 2048]
    # wq/wk arrive mt-major so the first head-pair's weights are one
    # contiguous run and can land first
    wq_m = wq_d.rearrange("p (m k q) -> p m k q", m=MT, k=KT)
    wk_m = wk_d.rearrange("p (m k q) -> p m k q", m=MT, k=KT)
    wv_k = wv_d.rearrange("p (k m) -> p k m", k=KT)
    wo_k = wo_d.rearrange("p (k e) -> p k e", k=MT)      # [128, 4, 1024]

    with tile.TileContext(nc) as tc, ExitStack() as ctx:
        persist = ctx.enter_context(tc.tile_pool(name="persist", bufs=1))

        qt = [persist.tile([P, T], BF16, name=f"qt{i}") for i in range(MT)]
        kt_ = [persist.tile([P, T], BF16, name=f"kt{i}") for i in range(MT)]
        v3 = [persist.tile([P, NH, HD + 1], BF16, name=f"v3_{i}")
              for i in range(TT)]
        at = [persist.tile([P, T], BF16, name=f"at{i}") for i in range(MT)]
        xt_all = persist.tile([P, KT, T], BF16, name="xt")

        wq_sb = persist.tile([P, MT, KT, P], BF16, name="wq")
        wk_sb = persist.tile([P, MT, KT, P], BF16, name="wk")
        bq_sb = persist.tile([P, MT], F32, name="bq")
        bk_sb = persist.tile([P, MT], F32, name="bk")
        bvb_sb = persist.tile([P, NH, HD], F32, name="bvb")
        tri_sb = persist.tile([P, P], BF16, name="tri")
        wv_sb = persist.tile([P, KT, QD], BF16, name="wv")
        wo_sb = persist.tile([P, MT, D], BF16, name="wo")

        # Startup DMAs spread over the three DMA-capable queues (sync/SP,
        # scalar/Activation, gpsimd; ~90 GB/s each) with deadline-aware
        # order: wq/wk head-pair 0 + the first x chunk (k-halves split
        # across two queues) land ~12us in; x chunk 1 by ~20us.
        nc.scalar.dma_start(bq_sb[:], bq_d)
        nc.scalar.dma_start(wq_sb[:, 0], wq_m[:, 0])
        nc.scalar.dma_start(xt_all[:, 4:8, 0:ACH], xT_k[:, 4:8, 0:ACH])
        nc.scalar.dma_start(wq_sb[:, 1], wq_m[:, 1])
        nc.scalar.dma_start(
            bvb_sb[:], bvb_d.rearrange("p (h d) -> p h d", d=HD))
        nc.scalar.dma_start(xt_all[:, 4:8, ACH:2 * ACH],
                            xT_k[:, 4:8, ACH:2 * ACH])
        nc.scalar.dma_start(wq_sb[:, 2:4], wq_m[:, 2:4])
        nc.gpsimd.dma_start(bk_sb[:], bk_d)
        nc.gpsimd.dma_start(wk_sb[:, 0], wk_m[:, 0])
        nc.gpsimd.dma_start(wv_sb[:], wv_k)
        nc.gpsimd.dma_start(tri_sb[:], msk_d)
        nc.gpsimd.dma_start(wk_sb[:, 1], wk_m[:, 1])
        nc.gpsimd.dma_start(wk_sb[:, 2:4], wk_m[:, 2:4])
        nc.gpsimd.dma_start(wo_sb[:], wo_k)
        nc.sync.dma_start(xt_all[:, 0:4, 0:ACH], xT_k[:, 0:4, 0:ACH])
        nc.sync.dma_start(xt_all[:, 0:4, ACH:2 * ACH],
                          xT_k[:, 0:4, ACH:2 * ACH])
        nc.sync.dma_start(xt_all[:, :, 2 * ACH:3 * ACH],
                          xT_k[:, :, 2 * ACH:3 * ACH])
        nc.sync.dma_start(xt_all[:, :, 3 * ACH:4 * ACH],
                          xT_k[:, :, 3 * ACH:4 * ACH])
        for tt in range(TT):
            nc.vector.memset(v3[tt][:, :, HD:HD + 1], 1.0)

        # ---- runway: Q/K for (chunk 0, head-pair 0) in a short-lived pool
        # whose banks free up before the attention pools open ------------
        with tc.tile_pool(name="rway", bufs=1, space="PSUM") as rp:
            for w_sb, dst, b_sb in ((wq_sb, qt, bq_sb), (wk_sb, kt_, bk_sb)):
                ps = rp.tile([P, ACH], F32, name="rw", bufs=2)
                for k in range(KT):
                    nc.tensor.matmul(ps[:], w_sb[:, 0, k, :],
                                     xt_all[:, k, 0:ACH],
                                     start=(k == 0), stop=(k == KT - 1))
                nc.vector.tensor_scalar_add(dst[0][:, 0:ACH], ps[:],
                                            b_sb[:, 0:1])

        # ---- attention + fillers, one fused software-pipelined stream ----
        with tc.tile_pool(name="attnsb", bufs=1) as ap_, \
             tc.tile_pool(name="obp", bufs=3) as obp, \
             tc.tile_pool(name="attnps", bufs=1, space="PSUM") as sp:

            def emit_projqk(c, hp, which):
                """Project Q (which=0) or K (which=1) for (chunk c, hp)."""
                w_sb, dst, b_sb = ((wq_sb, qt, bq_sb) if which == 0
                                   else (wk_sb, kt_, bk_sb))
                ps = sp.tile([P, ACH], F32, name="misc", bufs=1)
                csl = slice(c * ACH, (c + 1) * ACH)
                for k in range(KT):
                    nc.tensor.matmul(ps[:], w_sb[:, hp, k, :],
                                     xt_all[:, k, csl],
                                     start=(k == 0), stop=(k == KT - 1))
                nc.vector.tensor_scalar_add(dst[hp][:, csl], ps[:],
                                            b_sb[:, hp:hp + 1])

            def emit_v_tile(tt):
                psv = sp.tile([P, QD], F32, name="misc", bufs=1)
                for k in range(KT):
                    nc.tensor.matmul(
                        psv[:], xt_all[:, k, tt * P:(tt + 1) * P],
                        wv_sb[:, k, :], start=(k == 0), stop=(k == KT - 1))
                nc.vector.tensor_tensor(
                    v3[tt][:, :, 0:HD],
                    psv[:].rearrange("p (h d) -> p h d", d=HD),
                    bvb_sb[:], op=ALU.add)

            def emit_out_group(mt, nch2, alt=False):
                if alt:
                    # tail-only: borrow an (idle by then) spsum-tag slot so
                    # consecutive groups double-buffer instead of serializing
                    pso = sp.tile([P, 2 * ICH], F32, name="spsum",
                                  bufs=2)[:, 0:512]
                else:
                    pso = sp.tile([P, 512], F32, name="misc", bufs=1)
                for k in range(MT):
                    nc.tensor.matmul(
                        pso[:], at[k][:, mt * P:(mt + 1) * P],
                        wo_sb[:, k, nch2 * 512:(nch2 + 1) * 512],
                        start=(k == 0), stop=(k == MT - 1))
                ob = obp.tile([P, 512], F32, name="ob")
                nc.vector.tensor_copy(ob[:], pso[:])
                nc.sync.dma_start(
                    out_d[mt * P:(mt + 1) * P,
                          nch2 * 512:(nch2 + 1) * 512], ob[:])

            # ---- flattened attention pipeline over interleaved units ----
            # Units are (chunk, head-pair) blocks ordered so exp-heavy
            # later-chunk units pull forward into the projection-heavy
            # opening, keeping the scalar engine fed. The j-tile stream is
            # software-pipelined S one ahead, globally, across unit
            # boundaries.
            units = [(0, 0), (0, 1), (0, 2), (1, 0), (0, 3), (1, 1),
                     (1, 2), (2, 0), (1, 3), (2, 1), (2, 2), (3, 0),
                     (2, 3), (3, 1), (3, 2), (3, 3)]
            stream = [(ic, hp, jt) for (ic, hp) in units
                      for jt in range(4 * ic + 4)]

            s2s, e2s, opst = {}, {}, {}
            pending = []

            def emit_s(ic, hp, jt):
                s2 = sp.tile([P, 2 * ICH], F32, name="spsum", bufs=2)
                jsl = slice(jt * P, (jt + 1) * P)
                c0 = max(jt - 4 * ic, 0) * P
                qsl = slice(ic * ICH + c0, (ic + 1) * ICH)
                nc.tensor.matmul(s2[:, c0:ICH], kt_[hp][0:HD, jsl],
                                 qt[hp][0:HD, qsl], start=True, stop=True)
                nc.tensor.matmul(s2[:, ICH + c0:2 * ICH],
                                 kt_[hp][HD:P, jsl],
                                 qt[hp][HD:P, qsl], start=True, stop=True)
                s2s[(ic, hp, jt)] = s2

            def emit_exp(ic, hp, jt):
                e2 = ap_.tile([P, 2 * ICH], BF16, name="e", bufs=3)
                s2 = s2s.pop((ic, hp, jt))
                kdiag = jt - 4 * ic
                c0 = max(kdiag, 0) * P
                if kdiag == 3:
                    # two small valid ranges; split beats one span
                    nc.scalar.activation(e2[:, c0:ICH], s2[:, c0:ICH],
                                         AF.Exp)
                    nc.scalar.activation(e2[:, ICH + c0:2 * ICH],
                                         s2[:, ICH + c0:2 * ICH], AF.Exp)
                else:
                    # single span from first valid col of head A to the
                    # end; covers head B's dead cols but one ACT's fixed
                    # overhead beats two for small c0
                    nc.scalar.activation(e2[:, c0:2 * ICH],
                                         s2[:, c0:2 * ICH], AF.Exp)
                if kdiag >= 0:
                    # zero the diagonal block's upper triangle
                    for half in range(2):
                        o = half * ICH + c0
                        nc.vector.tensor_tensor(
                            e2[:, o:o + P], e2[:, o:o + P],
                            tri_sb[:], op=ALU.mult)
                e2s[(ic, hp, jt)] = e2

            def emit_av(ic, hp, jt):
                need(("V", jt))
                njt = 4 * ic + 4
                if jt == 0:
                    opst[(ic, hp)] = (
                        sp.tile([HD + 1, ICH], F32, name="opsum", bufs=3),
                        sp.tile([HD + 1, ICH], F32, name="opsum", bufs=3))
                opsA, opsB = opst[(ic, hp)]
                kdiag = jt - 4 * ic
                c0 = max(kdiag, 0) * P
                e2 = e2s.pop((ic, hp, jt))
                nc.tensor.matmul(opsA[:, c0:], v3[jt][:, 2 * hp, :],
                                 e2[:, c0:ICH],
                                 start=(jt == 0), stop=(jt == njt - 1))
                nc.tensor.matmul(opsB[:, c0:], v3[jt][:, 2 * hp + 1, :],
                                 e2[:, ICH + c0:2 * ICH],
                                 start=(jt == 0), stop=(jt == njt - 1))
                if jt == njt - 1:
                    isl = slice(ic * ICH, (ic + 1) * ICH)
                    opsA, opsB = opst.pop((ic, hp))

                    def normalize(hp=hp, isl=isl, opsA=opsA, opsB=opsB):
                        # first copy both accumulators (incl. the ones-row
                        # denominators) out of PSUM so the banks free for
                        # the next unit's AV; the rest of the chain runs
                        # from SBUF
                        ots = []
                        for ops in (opsA, opsB):
                            ot = ap_.tile([HD + 1, ICH], F32, name="ot",
                                          bufs=4)
                            nc.vector.tensor_copy(ot[:], ops[:])
                            ots.append(ot)
                        for half, ot in enumerate(ots):
                            po = half * HD
                            dn = ap_.tile([1, ICH], F32, name="dn", bufs=4)
                            nc.vector.tensor_copy(dn[:], ot[HD:HD + 1, :])
                            recip = ap_.tile([1, ICH], F32, name="recip",
                                             bufs=4)
                            nc.vector.reciprocal_approx_fast(recip[:], dn[:])
                            rb = ap_.tile([HD, ICH], F32, name="rb", bufs=4)
                            nc.gpsimd.partition_broadcast(rb[:], recip[:])
                            nc.vector.tensor_tensor(
                                at[hp][po:po + HD, isl], ot[0:HD, :], rb[:],
                                op=ALU.mult)

                    pending.append(normalize)

            # ---- global filler schedule: (due_slot, tag, fn) ------------
            def FQK(c, hp, which):
                return (("QK", c, hp, which),
                        lambda: emit_projqk(c, hp, which))

            def FV(tt):
                return (("V", tt), lambda: emit_v_tile(tt))

            def FO(mt, n):
                return (None, lambda: emit_out_group(mt, n))

            sched = [
                (0, FV(0)), (0, FV(1)), (1, FQK(0, 1, 0)),
                (2, FQK(0, 1, 1)), (2, FV(2)), (3, FV(3)),
                (7, FQK(0, 2, 0)), (8, FQK(0, 2, 1)),
                (9, FQK(1, 0, 0)), (10, FQK(1, 0, 1)),
                (12, FV(4)), (13, FV(5)), (14, FV(6)), (15, FV(7)),
                (16, FQK(0, 3, 0)), (17, FQK(0, 3, 1)),
                (20, FQK(1, 1, 0)), (21, FQK(1, 1, 1)),
                (25, FQK(1, 2, 0)), (28, FQK(1, 2, 1)),
                (33, FQK(2, 0, 0)), (36, FQK(2, 0, 1)),
                (42, FV(8)), (43, FV(9)), (44, FQK(1, 3, 0)),
                (45, FV(10)), (46, FV(11)), (47, FQK(1, 3, 1)),
                (54, FQK(2, 1, 0)), (57, FQK(2, 1, 1)),
                (60, FO(0, 0)), (63, FO(0, 1)),
                (64, FQK(2, 2, 0)), (66, FO(1, 0)), (67, FQK(2, 2, 1)),
                (69, FO(1, 1)), (72, FO(2, 0)),
                (74, FQK(3, 0, 0)), (75, FO(2, 1)),
                (78, FQK(3, 0, 1)), (78, FO(3, 0)), (81, FO(3, 1)),
                (88, FQK(2, 3, 0)), (89, FV(12)), (91, FV(13)),
                (92, FQK(2, 3, 1)), (93, FV(14)), (95, FV(15)),
                (100, FO(4, 0)), (102, FQK(3, 1, 0)), (104, FO(4, 1)),
                (106, FQK(3, 1, 1)), (108, FO(5, 0)), (112, FO(5, 1)),
                (116, FQK(3, 2, 0)), (116, FO(6, 0)), (120, FO(6, 1)),
                (120, FQK(3, 2, 1)), (124, FO(7, 0)), (126, FO(7, 1)),
                (128, FO(8, 0)), (131, FO(8, 1)),
                (132, FQK(3, 3, 0)), (134, FO(9, 0)),
                (136, FQK(3, 3, 1)), (137, FO(9, 1)),
                (140, FO(10, 0)), (143, FO(10, 1)),
                (146, FO(11, 0)), (149, FO(11, 1)),
            ]
            dues = [d for d, _ in sched]
            fillers = [f for _, f in sched]
            tag_idx = {tag: i for i, (tag, _) in enumerate(fillers)
                       if tag is not None}
            drained = [0]

            def drain_to(i):
                while drained[0] <= i:
                    fillers[drained[0]][1]()
                    drained[0] += 1

            def need(tag):
                if tag in tag_idx:
                    drain_to(tag_idx[tag])

            def maybe_fill(slot):
                while (drained[0] < len(fillers)
                       and dues[drained[0]] <= slot):
                    fillers[drained[0]][1]()
                    drained[0] += 1

            # ---- run the stream -------------------------------------
            for idx, (ic, hp, jt) in enumerate(stream):
                if jt == 0:
                    need(("QK", ic, hp, 0))
                    need(("QK", ic, hp, 1))
                if jt == 1 and pending:
                    pending.pop(0)()
                emit_s(ic, hp, jt)
                if idx >= 1:
                    emit_exp(*stream[idx - 1])
                    emit_av(*stream[idx - 1])
                maybe_fill(idx)
            emit_exp(*stream[-1])
            emit_av(*stream[-1])
            while pending:
                pending.pop(0)()
            drain_to(len(fillers) - 1)
            for i, (mt, n) in enumerate(
                    (mt, n) for mt in range(12, 16) for n in range(2)):
                emit_out_group(mt, n, alt=(i % 2 == 1))

    nc.compile()
    return nc


def _get_program():
    global _PROGRAM
    if _PROGRAM is None:
        _install_ntff_hook()
        _PROGRAM = _build_program()
    return _PROGRAM


def _make_masks():
    """Multiplicative upper-triangle zero mask [128, 128] for the diagonal
    128x128 block of each S^T tile: entry (j, i) = 1 if j <= i else 0."""
    j = np.arange(P)[:, None]
    i = np.arange(P)[None, :]
    return (j <= i).astype(np.float32)


def make_in_maps(x, Wq, bq, Wk, bk, Wv, bv, Wo, bo):
    import ml_dtypes
    bf16 = ml_dtypes.bfloat16

    def sbl(a, k):
        """[k*128, n] -> SBUF layout [128, k*n] (partition-major runs)."""
        n = a.shape[1]
        return np.ascontiguousarray(
            a.reshape(k, P, n).transpose(1, 0, 2).reshape(P, k * n)
        ).astype(bf16)

    def sbl_mt(a):
        """[1024, 512] weight -> mt-major SBUF layout [128, MT*KT*128]."""
        return np.ascontiguousarray(
            a.reshape(KT, P, MT, P).transpose(1, 2, 0, 3).reshape(P, -1)
        ).astype(bf16)

    masks = _make_masks()
    in_maps = []
    for c in range(8):
        b, hg = c // 2, c % 2
        sl = slice(hg * QD, (hg + 1) * QD)
        in_maps.append({
            "xT": sbl(np.ascontiguousarray(x[b].T), KT),
            "wq": sbl_mt(Wq[:, sl] * SCALE),
            "wk": sbl_mt(Wk[:, sl]),
            "wv": sbl(Wv[:, sl], KT),
            "wo": sbl(Wo[sl, :], MT),
            "bq": np.ascontiguousarray((bq[sl] * SCALE).reshape(MT, P).T),
            "bk": np.ascontiguousarray(bk[sl].reshape(MT, P).T),
            "bvb": np.ascontiguousarray(
                np.broadcast_to(bv[sl].astype(np.float32), (P, QD))),
            "msk": masks.astype(bf16),
        })
    return in_maps


def run(inputs, trace=False):
    from concourse.bass_utils import run_bass_kernel_spmd

    nc = _get_program()
    in_maps = make_in_maps(**inputs)
    res = run_bass_kernel_spmd(nc, in_maps, list(range(8)), trace=trace)
    bo = inputs["bo"]
    out = np.empty((B, T, D), dtype=np.float32)
    for b in range(B):
        out[b] = res.results[2 * b]["out"] + res.results[2 * b + 1]["out"] + bo
    return out, res


def kernel(**inputs):
    inputs = {k: np.asarray(v) for k, v in inputs.items()}
    out, _ = run(inputs)
    return out


# revision 15
# speedup vs baseline: 1.0816x; 1.0816x over previous
"""Causal self-attention (B=4, T=2048, D=1024, H=16, hd=64) on 8 trn2 NeuronCores.

Sharding: data parallel over batch (4) x tensor parallel over heads (2 groups
of 8). Core c handles batch c//2 and heads (c%2)*8 .. (c%2)*8+8.
Wq/Wk/Wv are column-parallel by head group, Wo row-parallel; the pair of
cores sharing a batch produce partial outputs that are summed on the host.

On-device layout (per core) is fully "transposed": projections produce
Q^T, K^T [512, 2048] and V [2048, 512], scores are computed as
S^T = K Q^T (j=key on partitions, i=query on free dim), softmax uses
exp without max subtraction (scores are O(6) here), the denominator
comes for free from a ones-column appended to V, and attention output
O^T [hd, T] feeds the row-parallel out-projection directly as lhsT.

v2 schedule: the scalar engine's exp stream (160 x ~1.1us) is the pacer.
Everything else (Q/K/V projections, out-projection) is emitted as fillers
inside the attention j-tile loop so exp starts ~10us into the run and the
PE never idles waiting for it. Exp and the S matmuls are trimmed to the
causally-valid query range of each diagonal tile.
"""

import contextlib
import ctypes
import sys
import types

import numpy as np

B, T, D = 4, 2048, 1024
H_TOT, HD = 16, 64
SCALE = HD ** -0.5
P = 128
NH = 8            # heads per core
QD = NH * HD      # 512, projected dim per core
KT = D // P       # 8 contraction tiles for projections
MT = QD // P      # 4 qdim tiles
TT = T // P       # 16 token tiles
ACH = 512         # token chunk; PSUM bank caps matmul N at 512
NACH = T // ACH   # 4
ICH = 512         # attention query chunk
NIC = T // ICH    # 4

_PROGRAM = None  # compiled program cache — build once per process


def _install_ntff_hook():
    """antenv.axon_hooks is missing in this image; recreate it so
    run_bass_kernel_spmd(trace=True) can profile. Harmless if unused."""
    if "antenv.axon_hooks" in sys.modules:
        return
    try:
        import antenv
    except ImportError:
        return
    mod = types.ModuleType("antenv.axon_hooks")
    _hook = [None]
    mod.set_axon_ntff_profile_hook = lambda h: _hook.__setitem__(0, h)
    mod.get_axon_ntff_profile_hook = lambda: _hook[0]
    antenv.axon_hooks = mod
    sys.modules["antenv.axon_hooks"] = mod
    try:
        lib = ctypes.CDLL("/opt/axon/libaxon_pjrt.so")
        if not hasattr(lib, "axon_start_nrt_profile"):
            return
        lib.axon_start_nrt_profile.argtypes = [
            ctypes.POINTER(ctypes.c_int64), ctypes.c_size_t]
        lib.axon_start_nrt_profile.restype = ctypes.c_int64
        lib.axon_stop_nrt_profile.argtypes = [ctypes.c_char_p]
        lib.axon_stop_nrt_profile.restype = ctypes.c_int64

        @contextlib.contextmanager
        def _hookfn(output_dir, device_ids):
            import jax
            jax.devices()
            if device_ids:
                ids = (ctypes.c_int64 * len(device_ids))(*device_ids)
                rc = lib.axon_start_nrt_profile(ids, len(device_ids))
            else:
                rc = lib.axon_start_nrt_profile(None, 0)
            if rc != 0:
                raise RuntimeError(f"axon_start_nrt_profile rc={rc}")
            try:
                yield
            finally:
                n = lib.axon_stop_nrt_profile(str(output_dir).encode())
                print(f"profile: {n} file(s) written to {output_dir}")

        mod.set_axon_ntff_profile_hook(_hookfn)
    except OSError:
        pass


def _build_program():
    from contextlib import ExitStack

    import concourse.tile as tile
    from concourse import bacc, mybir

    F32 = mybir.dt.float32
    BF16 = mybir.dt.bfloat16
    AF = mybir.ActivationFunctionType
    ALU = mybir.AluOpType

    nc = bacc.Bacc("TRN2", target_bir_lowering=False, debug=False,
                   num_devices=8)

    # all tensor inputs arrive pre-arranged in SBUF layout [128, k, n]
    # (host does the transpose) so every DMA is long contiguous runs
    xT_d = nc.dram_tensor("xT", [P, KT * T], BF16, kind="ExternalInput").ap()
    wq_d = nc.dram_tensor("wq", [P, KT * QD], BF16, kind="ExternalInput").ap()
    wk_d = nc.dram_tensor("wk", [P, KT * QD], BF16, kind="ExternalInput").ap()
    wv_d = nc.dram_tensor("wv", [P, KT * QD], BF16, kind="ExternalInput").ap()
    wo_d = nc.dram_tensor("wo", [P, MT * D], BF16, kind="ExternalInput").ap()
    bq_d = nc.dram_tensor("bq", [P, MT], F32, kind="ExternalInput").ap()
    bk_d = nc.dram_tensor("bk", [P, MT], F32, kind="ExternalInput").ap()
    bvb_d = nc.dram_tensor("bvb", [P, QD], F32, kind="ExternalInput").ap()
    msk_d = nc.dram_tensor("msk", [P, P], BF16, kind="ExternalInput").ap()
    out_d = nc.dram_tensor("out", [T, D], F32, kind="ExternalOutput").ap()

    xT_k = xT_d.rearrange("p (k t) -> p k t", k=KT)      # [128, 8, 2048]
    # wq/wk arrive mt-major so the first head-pair's weights are one
    # contiguous run and can land first
    wq_m = wq_d.rearrange("p (m k q) -> p m k q", m=MT, k=KT)
    wk_m = wk_d.rearrange("p (m k q) -> p m k q", m=MT, k=KT)
    wv_k = wv_d.rearrange("p (k m) -> p k m", k=KT)
    wo_k = wo_d.rearrange("p (k e) -> p k e", k=MT)      # [128, 4, 1024]

    with tile.TileContext(nc) as tc, ExitStack() as ctx:
        persist = ctx.enter_context(tc.tile_pool(name="persist", bufs=1))

        qt = [persist.tile([P, T], BF16, name=f"qt{i}") for i in range(MT)]
        kt_ = [persist.tile([P, T], BF16, name=f"kt{i}") for i in range(MT)]
        v3 = [persist.tile([P, NH, HD + 1], BF16, name=f"v3_{i}")
              for i in range(TT)]
        at = [persist.tile([P, T], BF16, name=f"at{i}") for i in range(MT)]
        xt_all = persist.tile([P, KT, T], BF16, name="xt")

        wq_sb = persist.tile([P, MT, KT, P], BF16, name="wq")
        wk_sb = persist.tile([P, MT, KT, P], BF16, name="wk")
        bq_sb = persist.tile([P, MT], F32, name="bq")
        bk_sb = persist.tile([P, MT], F32, name="bk")
        bvb_sb = persist.tile([P, NH, HD], F32, name="bvb")
        tri_sb = persist.tile([P, P], BF16, name="tri")
        wv_sb = persist.tile([P, KT, QD], BF16, name="wv")
        wo_sb = persist.tile([P, MT, D], BF16, name="wo")

        # Startup DMAs spread over the three DMA-capable queues (sync/SP,
        # scalar/Activation, gpsimd; ~90 GB/s each) with deadline-aware
        # order: wq/wk head-pair 0 + the first x chunk (k-halves split
        # across two queues) land ~12us in; x chunk 1 by ~20us.
        nc.scalar.dma_start(bq_sb[:], bq_d)
        nc.scalar.dma_start(wq_sb[:, 0], wq_m[:, 0])
        nc.scalar.dma_start(wq_sb[:, 1], wq_m[:, 1])
        nc.scalar.dma_start(
            bvb_sb[:], bvb_d.rearrange("p (h d) -> p h d", d=HD))
        nc.scalar.dma_start(wq_sb[:, 2:4], wq_m[:, 2:4])
        nc.gpsimd.dma_start(bk_sb[:], bk_d)
        nc.gpsimd.dma_start(wk_sb[:, 0], wk_m[:, 0])
        nc.gpsimd.dma_start(tri_sb[:], msk_d)
        nc.gpsimd.dma_start(wv_sb[:], wv_k)
        nc.gpsimd.dma_start(wk_sb[:, 1], wk_m[:, 1])
        nc.gpsimd.dma_start(wk_sb[:, 2:4], wk_m[:, 2:4])
        nc.gpsimd.dma_start(wo_sb[:], wo_k)
        nc.sync.dma_start(xt_all[:, 0:4, 0:ACH], xT_k[:, 0:4, 0:ACH])
        nc.sync.dma_start(xt_all[:, 4:8, 0:ACH], xT_k[:, 4:8, 0:ACH])
        nc.sync.dma_start(xt_all[:, :, ACH:2 * ACH], xT_k[:, :, ACH:2 * ACH])
        nc.sync.dma_start(xt_all[:, :, 2 * ACH:3 * ACH],
                          xT_k[:, :, 2 * ACH:3 * ACH])
        nc.sync.dma_start(xt_all[:, :, 3 * ACH:4 * ACH],
                          xT_k[:, :, 3 * ACH:4 * ACH])
        for tt in range(TT):
            nc.vector.memset(v3[tt][:, :, HD:HD + 1], 1.0)

        # ---- runway: Q/K for (chunk 0, head-pair 0) in a short-lived pool
        # whose banks free up before the attention pools open ------------
        with tc.tile_pool(name="rway", bufs=1, space="PSUM") as rp:
            for w_sb, dst, b_sb in ((wq_sb, qt, bq_sb), (wk_sb, kt_, bk_sb)):
                ps = rp.tile([P, ACH], F32, name="rw", bufs=2)
                for k in range(KT):
                    nc.tensor.matmul(ps[:], w_sb[:, 0, k, :],
                                     xt_all[:, k, 0:ACH],
                                     start=(k == 0), stop=(k == KT - 1))
                nc.vector.tensor_scalar_add(dst[0][:, 0:ACH], ps[:],
                                            b_sb[:, 0:1])

        # ---- attention + fillers, one fused software-pipelined stream ----
        with tc.tile_pool(name="attnsb", bufs=1) as ap_, \
             tc.tile_pool(name="obp", bufs=3) as obp, \
             tc.tile_pool(name="attnps", bufs=1, space="PSUM") as sp:

            half_ps = {}

            def emit_projqk(c, hp, which, half):
                """Half a Q/K projection for (chunk c, hp): 4 k-steps per
                drain so a filler never delays the next S-pair enough to
                starve the exp stream."""
                w_sb, dst, b_sb = ((wq_sb, qt, bq_sb) if which == 0
                                   else (wk_sb, kt_, bk_sb))
                key = ("qk", c, hp, which)
                if half == 0:
                    half_ps[key] = sp.tile([P, ACH], F32, name="misc",
                                           bufs=1)
                ps = half_ps[key]
                csl = slice(c * ACH, (c + 1) * ACH)
                for k in range(4 * half, 4 * half + 4):
                    nc.tensor.matmul(ps[:], w_sb[:, hp, k, :],
                                     xt_all[:, k, csl],
                                     start=(k == 0), stop=(k == KT - 1))
                if half == 1:
                    del half_ps[key]
                    nc.vector.tensor_scalar_add(dst[hp][:, csl], ps[:],
                                                b_sb[:, hp:hp + 1])

            def emit_v_tile(tt, half):
                key = ("v", tt)
                if half == 0:
                    half_ps[key] = sp.tile([P, QD], F32, name="misc",
                                           bufs=1)
                psv = half_ps[key]
                for k in range(4 * half, 4 * half + 4):
                    nc.tensor.matmul(
                        psv[:], xt_all[:, k, tt * P:(tt + 1) * P],
                        wv_sb[:, k, :], start=(k == 0), stop=(k == KT - 1))
                if half == 1:
                    del half_ps[key]
                    nc.vector.tensor_tensor(
                        v3[tt][:, :, 0:HD],
                        psv[:].rearrange("p (h d) -> p h d", d=HD),
                        bvb_sb[:], op=ALU.add)

            def emit_out_group(mt, nch2, alt=False):
                if alt:
                    # tail-only: borrow an (idle by then) spsum-tag slot so
                    # consecutive groups double-buffer instead of serializing
                    pso = sp.tile([P, 2 * ICH], F32, name="spsum",
                                  bufs=2)[:, 0:512]
                else:
                    pso = sp.tile([P, 512], F32, name="misc", bufs=1)
                for k in range(MT):
                    nc.tensor.matmul(
                        pso[:], at[k][:, mt * P:(mt + 1) * P],
                        wo_sb[:, k, nch2 * 512:(nch2 + 1) * 512],
                        start=(k == 0), stop=(k == MT - 1))
                ob = obp.tile([P, 512], F32, name="ob")
                nc.vector.tensor_copy(ob[:], pso[:])
                nc.sync.dma_start(
                    out_d[mt * P:(mt + 1) * P,
                          nch2 * 512:(nch2 + 1) * 512], ob[:])

            # ---- flattened attention pipeline over interleaved units ----
            # Units are (chunk, head-pair) blocks ordered so exp-heavy
            # later-chunk units pull forward into the projection-heavy
            # opening, keeping the scalar engine fed. The j-tile stream is
            # software-pipelined S one ahead, globally, across unit
            # boundaries.
            units = [(0, 0), (0, 1), (0, 2), (1, 0), (0, 3), (1, 1),
                     (1, 2), (2, 0), (1, 3), (2, 1), (2, 2), (3, 0),
                     (2, 3), (3, 1), (3, 2), (3, 3)]
            stream = [(ic, hp, jt) for (ic, hp) in units
                      for jt in range(4 * ic + 4)]

            s2s, e2s, opst = {}, {}, {}
            pending = []

            def emit_s(ic, hp, jt):
                s2 = sp.tile([P, 2 * ICH], F32, name="spsum", bufs=2)
                jsl = slice(jt * P, (jt + 1) * P)
                c0 = max(jt - 4 * ic, 0) * P
                qsl = slice(ic * ICH + c0, (ic + 1) * ICH)
                nc.tensor.matmul(s2[:, c0:ICH], kt_[hp][0:HD, jsl],
                                 qt[hp][0:HD, qsl], start=True, stop=True)
                nc.tensor.matmul(s2[:, ICH + c0:2 * ICH],
                                 kt_[hp][HD:P, jsl],
                                 qt[hp][HD:P, qsl], start=True, stop=True)
                s2s[(ic, hp, jt)] = s2

            def emit_exp(ic, hp, jt):
                e2 = ap_.tile([P, 2 * ICH], BF16, name="e", bufs=4)
                s2 = s2s.pop((ic, hp, jt))
                kdiag = jt - 4 * ic
                c0 = max(kdiag, 0) * P
                if kdiag == 3:
                    # two small valid ranges; split beats one span
                    nc.scalar.activation(e2[:, c0:ICH], s2[:, c0:ICH],
                                         AF.Exp)
                    nc.scalar.activation(e2[:, ICH + c0:2 * ICH],
                                         s2[:, ICH + c0:2 * ICH], AF.Exp)
                else:
                    # single span from first valid col of head A to the
                    # end; covers head B's dead cols but one ACT's fixed
                    # overhead beats two for small c0
                    nc.scalar.activation(e2[:, c0:2 * ICH],
                                         s2[:, c0:2 * ICH], AF.Exp)
                if kdiag >= 0:
                    # zero the diagonal block's upper triangle
                    for half in range(2):
                        o = half * ICH + c0
                        nc.vector.tensor_tensor(
                            e2[:, o:o + P], e2[:, o:o + P],
                            tri_sb[:], op=ALU.mult)
                e2s[(ic, hp, jt)] = e2

            def emit_av(ic, hp, jt):
                need(("V", jt))
                njt = 4 * ic + 4
                if jt == 0:
                    opst[(ic, hp)] = (
                        sp.tile([HD + 1, ICH], F32, name="opsum", bufs=3),
                        sp.tile([HD + 1, ICH], F32, name="opsum", bufs=3))
                opsA, opsB = opst[(ic, hp)]
                kdiag = jt - 4 * ic
                c0 = max(kdiag, 0) * P
                e2 = e2s.pop((ic, hp, jt))
                nc.tensor.matmul(opsA[:, c0:], v3[jt][:, 2 * hp, :],
                                 e2[:, c0:ICH],
                                 start=(jt == 0), stop=(jt == njt - 1))
                nc.tensor.matmul(opsB[:, c0:], v3[jt][:, 2 * hp + 1, :],
                                 e2[:, ICH + c0:2 * ICH],
                                 start=(jt == 0), stop=(jt == njt - 1))
                if jt == njt - 1:
                    isl = slice(ic * ICH, (ic + 1) * ICH)
                    opsA, opsB = opst.pop((ic, hp))

                    def normalize(hp=hp, isl=isl, opsA=opsA, opsB=opsB):
                        # first copy both accumulators (incl. the ones-row
                        # denominators) out of PSUM so the banks free for
                        # the next unit's AV; the rest of the chain runs
                        # from SBUF
                        ots = []
                        for ops in (opsA, opsB):
                            ot = ap_.tile([HD + 1, ICH], F32, name="ot",
                                          bufs=4)
                            nc.vector.tensor_copy(ot[:], ops[:])
                            ots.append(ot)
                        for half, ot in enumerate(ots):
                            po = half * HD
                            dn = ap_.tile([1, ICH], F32, name="dn", bufs=4)
                            nc.vector.tensor_copy(dn[:], ot[HD:HD + 1, :])
                            recip = ap_.tile([1, ICH], F32, name="recip",
                                             bufs=4)
                            nc.vector.reciprocal_approx_fast(recip[:], dn[:])
                            rb = ap_.tile([HD, ICH], F32, name="rb", bufs=4)
                            nc.gpsimd.partition_broadcast(rb[:], recip[:])
                            nc.vector.tensor_tensor(
                                at[hp][po:po + HD, isl], ot[0:HD, :], rb[:],
                                op=ALU.mult)

                    pending.append(normalize)

            # ---- global filler schedule: (due_slot, tag, fn) ------------
            def FQK(c, hp, which, half):
                tag = ("QK", c, hp, which) if half == 1 else None
                return (tag, lambda: emit_projqk(c, hp, which, half))

            def FV(tt, half):
                tag = ("V", tt) if half == 1 else None
                return (tag, lambda: emit_v_tile(tt, half))

            def FO(mt, n):
                return (None, lambda: emit_out_group(mt, n))

            sched = []
            for args in [
                # (due, kind, params)  kind: q=projqk half, v=vtile half,
                # o=out group
                (0, "v", 0, 0), (0, "v", 0, 1), (0, "v", 1, 0),
                (1, "v", 1, 1),
                (1, "q", 0, 1, 0, 0), (2, "q", 0, 1, 0, 1),
                (2, "q", 0, 1, 1, 0), (3, "q", 0, 1, 1, 1),
                (3, "v", 2, 0), (4, "v", 2, 1),
                (5, "v", 3, 0), (5, "v", 3, 1),
                (6, "q", 0, 2, 0, 0), (6, "q", 0, 2, 0, 1),
                (7, "q", 0, 2, 1, 0), (7, "q", 0, 2, 1, 1),
                (8, "q", 1, 0, 0, 0), (9, "q", 1, 0, 0, 1),
                (10, "q", 1, 0, 1, 0), (11, "q", 1, 0, 1, 1),
                (12, "v", 4, 0), (13, "v", 4, 1),
                (14, "v", 5, 0), (15, "v", 5, 1),
                (16, "v", 6, 0), (16, "v", 6, 1),
                (17, "v", 7, 0), (17, "v", 7, 1),
                (18, "q", 0, 3, 0, 0), (18, "q", 0, 3, 0, 1),
                (19, "q", 0, 3, 1, 0), (19, "q", 0, 3, 1, 1),
                (22, "q", 1, 1, 0, 0), (23, "q", 1, 1, 0, 1),
                (23, "q", 1, 1, 1, 0), (24, "q", 1, 1, 1, 1),
                (28, "q", 1, 2, 0, 0), (29, "q", 1, 2, 0, 1),
                (30, "q", 1, 2, 1, 0), (31, "q", 1, 2, 1, 1),
                (36, "q", 2, 0, 0, 0), (37, "q", 2, 0, 0, 1),
                (38, "q", 2, 0, 1, 0), (39, "q", 2, 0, 1, 1),
                (44, "v", 8, 0), (45, "v", 8, 1),
                (46, "v", 9, 0), (46, "v", 9, 1),
                (47, "v", 10, 0), (47, "v", 10, 1),
                (48, "v", 11, 0), (48, "v", 11, 1),
                (49, "q", 1, 3, 0, 0), (50, "q", 1, 3, 0, 1),
                (50, "q", 1, 3, 1, 0), (51, "q", 1, 3, 1, 1),
                (56, "q", 2, 1, 0, 0), (57, "q", 2, 1, 0, 1),
                (58, "q", 2, 1, 1, 0), (59, "q", 2, 1, 1, 1),
                (62, "o", 0, 0), (65, "o", 0, 1),
                (68, "o", 1, 0), (71, "o", 1, 1),
                (70, "q", 2, 2, 0, 0), (70, "q", 2, 2, 0, 1),
                (71, "q", 2, 2, 1, 0), (71, "q", 2, 2, 1, 1),
                (74, "o", 2, 0), (77, "o", 2, 1),
                (80, "q", 3, 0, 0, 0), (81, "q", 3, 0, 0, 1),
                (82, "q", 3, 0, 1, 0), (83, "q", 3, 0, 1, 1),
                (84, "o", 3, 0), (86, "o", 3, 1),
                (88, "v", 12, 0), (89, "v", 12, 1),
                (90, "v", 13, 0), (91, "v", 13, 1),
                (92, "v", 14, 0), (93, "v", 14, 1),
                (94, "v", 15, 0), (95, "v", 15, 1),
                (96, "q", 2, 3, 0, 0), (97, "q", 2, 3, 0, 1),
                (98, "q", 2, 3, 1, 0), (99, "q", 2, 3, 1, 1),
                (101, "o", 4, 0), (104, "o", 4, 1),
                (107, "o", 5, 0), (110, "o", 5, 1),
                (108, "q", 3, 1, 0, 0), (109, "q", 3, 1, 0, 1),
                (110, "q", 3, 1, 1, 0), (111, "q", 3, 1, 1, 1),
                (114, "o", 6, 0), (117, "o", 6, 1),
                (120, "o", 7, 0), (123, "o", 7, 1),
                (124, "q", 3, 2, 0, 0), (125, "q", 3, 2, 0, 1),
                (126, "q", 3, 2, 1, 0), (127, "q", 3, 2, 1, 1),
                (130, "o", 8, 0), (133, "o", 8, 1),
                (136, "o", 9, 0), (139, "o", 9, 1),
                (140, "q", 3, 3, 0, 0), (141, "q", 3, 3, 0, 1),
                (142, "q", 3, 3, 1, 0), (143, "q", 3, 3, 1, 1),
                (146, "o", 10, 0), (149, "o", 10, 1),
                (152, "o", 11, 0), (155, "o", 11, 1),
            ]:
                due, kind = args[0], args[1]
                if kind == "q":
                    c, hp, which, half = args[2:]
                    sched.append((due, FQK(c, hp, which, half)))
                elif kind == "v":
                    tt, half = args[2:]
                    sched.append((due, FV(tt, half)))
                else:
                    mt, n2 = args[2:]
                    sched.append((due, FO(mt, n2)))
            sched.sort(key=lambda x: x[0])
            dues = [d for d, _ in sched]
            fillers = [f for _, f in sched]
            tag_idx = {tag: i for i, (tag, _) in enumerate(fillers)
                       if tag is not None}
            drained = [0]

            def drain_to(i):
                while drained[0] <= i:
                    fillers[drained[0]][1]()
                    drained[0] += 1

            def need(tag):
                if tag in tag_idx:
                    drain_to(tag_idx[tag])

            def maybe_fill(slot):
                while (drained[0] < len(fillers)
                       and dues[drained[0]] <= slot):
                    fillers[drained[0]][1]()
                    drained[0] += 1

            # ---- run the stream -------------------------------------
            for idx, (ic, hp, jt) in enumerate(stream):
                if jt == 0:
                    need(("QK", ic, hp, 0))
                    need(("QK", ic, hp, 1))
                if jt == 3 and pending:
                    pending.pop(0)()
                emit_s(ic, hp, jt)
                if idx >= 1:
                    emit_exp(*stream[idx - 1])
                if idx >= 2:
                    emit_av(*stream[idx - 2])
                maybe_fill(idx)
            emit_exp(*stream[-1])
            emit_av(*stream[-2])
            emit_av(*stream[-1])
            while pending:
                pending.pop(0)()
            drain_to(len(fillers) - 1)
            for i, (mt, n) in enumerate(
                    (mt, n) for mt in range(12, 16) for n in range(2)):
                emit_out_group(mt, n, alt=(i % 2 == 1))

    nc.compile()
    return nc


def _get_program():
    global _PROGRAM
    if _PROGRAM is None:
        _install_ntff_hook()
        _PROGRAM = _build_program()
    return _PROGRAM


def _make_masks():
    """Multiplicative upper-triangle zero mask [128, 128] for the diagonal
    128x128 block of each S^T tile: entry (j, i) = 1 if j <= i else 0."""
    j = np.arange(P)[:, None]
    i = np.arange(P)[None, :]
    return (j <= i).astype(np.float32)


def make_in_maps(x, Wq, bq, Wk, bk, Wv, bv, Wo, bo):
    import ml_dtypes
    bf16 = ml_dtypes.bfloat16

    def sbl(a, k):
        """[k*128, n] -> SBUF layout [128, k*n] (partition-major runs)."""
        n = a.shape[1]
        return np.ascontiguousarray(
            a.reshape(k, P, n).transpose(1, 0, 2).reshape(P, k * n)
        ).astype(bf16)

    def sbl_mt(a):
        """[1024, 512] weight -> mt-major SBUF layout [128, MT*KT*128]."""
        return np.ascontiguousarray(
            a.reshape(KT, P, MT, P).transpose(1, 2, 0, 3).reshape(P, -1)
        ).astype(bf16)

    masks = _make_masks()
    in_maps = []
    for c in range(8):
        b, hg = c // 2, c % 2
        sl = slice(hg * QD, (hg + 1) * QD)
        in_maps.append({
            "xT": sbl(np.ascontiguousarray(x[b].T), KT),
            "wq": sbl_mt(Wq[:, sl] * SCALE),
            "wk": sbl_mt(Wk[:, sl]),
            "wv": sbl(Wv[:, sl], KT),
            "wo": sbl(Wo[sl, :], MT),
            "bq": np.ascontiguousarray((bq[sl] * SCALE).reshape(MT, P).T),
            "bk": np.ascontiguousarray(bk[sl].reshape(MT, P).T),
            "bvb": np.ascontiguousarray(
                np.broadcast_to(bv[sl].astype(np.float32), (P, QD))),
            "msk": masks.astype(bf16),
        })
    return in_maps


def run(inputs, trace=False):
    from concourse.bass_utils import run_bass_kernel_spmd

    nc = _get_program()
    in_maps = make_in_maps(**inputs)
    res = run_bass_kernel_spmd(nc, in_maps, list(range(8)), trace=trace)
    bo = inputs["bo"]
    out = np.empty((B, T, D), dtype=np.float32)
    for b in range(B):
        out[b] = res.results[2 * b]["out"] + res.results[2 * b + 1]["out"] + bo
    return out, res


def kernel(**inputs):
    inputs = {k: np.asarray(v) for k, v in inputs.items()}
    out, _ = run(inputs)
    return out
